# revision 36
# baseline (speedup 1.0000x reference)
"""Trainium2 Bass kernel for nn_Discriminator_80195629351349.

Pairwise-column MLP discriminator over k-space columns.

Math (matching the jax reference):
  F[b, w, ch] = |kspace[b, c, h, w]|  (ch = c*H + h)
  Pq = Fq @ W1[:, :CH].T ;  Pa = Fa @ W1[:, CH:].T          [B, W, 18]
  out[b, wi, wc] = sigmoid(W4 @ r3 + b4),  r3 = relu-chain of
                   relu(Pq[wi] + Pa[wc] + b1) through W2, W3
  heat[b, wi] = sum_wc out[b, wi, wc] * cmask[b, wc] / denom[b]
  result[b, h, w] = heat[b, w] if acquiring_mask[b, w] > 0 else 0

Only columns wi with acquiring_mask>0 (16 of 384) contribute to the
output, and the wc sum runs only over [left, right) (191 of 384), so the
kernel computes exactly that slice.

Sharding: 8 cores = (batch b in 0..3) x (wc half s in 0..1). Each core
gets a host-packed bf16 tensor of its k-space columns (wi + wc combined,
re/im planes separated, already in partition-major layout so every DMA
line is one contiguous descriptor per partition).

On-device pipeline (bf16 wire/compute, fp32 PSUM):
  DVE : sq = ks*ks, m2 = sq_re + sq_im       (dense 2x bf16 mode)
  ACT : f = sqrt(m2)                          (sqrt table prefetched)
  PE  : psum[36,112] += W1cat_k^T @ f_k       (24 bf16 matmuls)
  pair stage: Pa+b1 / Pq replicated into 4 partition quadrants via
  selector matmuls; relu stages on ACT (relu lives in every act table);
  final sigmoid on ACT with accum_out giving the wc-sum directly.
Host divides by denom and subtracts the padding-column constant.
"""

import math
import os

import numpy as np
import ml_dtypes

B, C, H, W = 4, 8, 384, 384
CH = C * H            # 3072 features per column
P = 128               # SBUF partitions
KT = CH // P          # 24 contraction tiles
CHANS = 18            # MLP width
NCORES = 8
BF16 = ml_dtypes.bfloat16

_prog_cache: dict = {}
LAST_RESULTS = None   # BassKernelResults of the most recent run (for test.py)


def _cst_layout(NWC: int, NS: int):
    """Column offsets in the bf16 constant block [128, CW]."""
    KC = KT * 50               # w1cat: 50 cols per k-tile (Pa@0, Pq@32)
    o = {}
    o["w1cat"] = 0
    o["w1end"] = KC                  # end of the w1 block
    o["sel"] = KC                    # [0:18, 128] quadrant replicator
    o["sel32"] = o["sel"] + P        # [32:50, 128] same eye, rows 32:50
    o["crest"] = o["sel"]            # start of the pair-stage constants
    o["w2"] = o["sel32"] + P         # [128, 128] block-diag
    o["w3"] = o["w2"] + P
    o["w4"] = o["w3"] + P            # [128, 4]
    o["b1p"] = o["w4"] + 4           # [1, 128] row 0: b1 quadrant pattern
    o["one"] = o["b1p"] + P          # [1, max(NWC,NL)] row 0: ones
    o["b2"] = o["one"] + max(NWC, 4 * ((NS // 4) or 1))
    o["b3"] = o["b2"] + 1
    o["b4"] = o["b3"] + 1            # [4, 1]
    o["end"] = o["b4"] + 1
    return o


def _build_program(NWC: int, NL: int):
    """SPMD Bass/Tile program for one core.

    NWC: wc (acquired) columns this core handles.
    NL:  wi slots per partition-quadrant (total wi slots = 4*NL).
    """
    import concourse.bass as bass
    import concourse.tile as tile
    from concourse import bacc, mybir

    f32 = mybir.dt.float32
    bf16 = mybir.dt.bfloat16
    NS = 4 * NL           # wi slots
    NC = NS + NWC         # combined columns per k-tile
    NF = NL * NWC         # free columns of the pair block
    CHUNKS = [4, 4, 4, 4, 4, 4]       # k-tiles per chunk
    NCHUNK = len(CHUNKS)
    assert sum(CHUNKS) == KT
    o = _cst_layout(NWC, NS)
    CW = o["end"]
    assert NF <= 512

    nc = bacc.Bacc("TRN2", debug=False)

    # ---- DRAM I/O ----
    # ks: [128, 2, KT*NC] bf16; plane 0 = re, plane 1 = im; within a
    # plane, k-tile-major then combined-column ([q cols | a cols]).
    ks = nc.dram_tensor("ks", [P, 2, KT * NC], bf16, kind="ExternalInput")
    cst = nc.dram_tensor("cst", [P, CW], bf16, kind="ExternalInput")
    hp = nc.dram_tensor("hp", [4, NL * NWC], bf16,
                        kind="ExternalOutput")

    AF = mybir.ActivationFunctionType
    ALU = mybir.AluOpType

    with tile.TileContext(nc) as tc:
        with (
            tc.tile_pool(name="consts", bufs=1) as consts,
            tc.tile_pool(name="kdata", bufs=1) as kdata,
            tc.tile_pool(name="sq", bufs=1) as sqp,
            tc.tile_pool(name="m2", bufs=1) as m2p,
            tc.tile_pool(name="feat", bufs=1) as feat,
            tc.tile_pool(name="mlp", bufs=1) as mlp,
            tc.tile_pool(name="sig", bufs=2) as sigp,
            tc.tile_pool(name="psum", bufs=1, space="PSUM") as psp,
        ):
            # table-priming tile: ready at t0 so ACT loads the sqrt set
            # while the first k-space chunk is still in flight
            prim = mlp.tile([1, 2], f32, tag="prim")
            nc.gpsimd.memset(prim, 1.0)
            nc.scalar.sqrt(prim[:, 1:2], prim[:, 0:1])

            # every DMA queue (sync/scalar HWDGE + gpsimd SWDGE) leads
            # with k-space chunks; w1 rides mid-queue; pair-stage
            # constants land last (they are only needed by the tail).
            cst_s = consts.tile([P, CW], bf16, tag="cst")
            W1E = o["w1end"]
            W1H = (W1E // 2) // 50 * 50
            CR = o["crest"]
            CR1 = o["w2"]
            b2_s = cst_s[:, o["b2"]:o["b2"] + 1]
            b3_s = cst_s[:, o["b3"]:o["b3"] + 1]
            b4_s = cst_s[0:4, o["b4"]:o["b4"] + 1]

            f_s = feat.tile([P, KT * NC], bf16, tag="f")
            psum1 = psp.tile([50, NC], f32, tag="p1")
            kcs = {}
            koff = [sum(CHUNKS[:i]) for i in range(NCHUNK + 1)]

            def ksdma(eng, ci):
                CL = CHUNKS[ci] * NC
                kc = kdata.tile([P, 2, CL], bf16, tag=f"kc{ci}")
                eng.dma_start(out=kc,
                              in_=ks[:, :, koff[ci] * NC:koff[ci + 1] * NC])
                kcs[ci] = kc

            ksdma(nc.scalar, 0)
            ksdma(nc.sync, 1)
            ksdma(nc.gpsimd, 2)
            nc.scalar.dma_start(out=cst_s[:, 0:W1H], in_=cst[:, 0:W1H])
            nc.sync.dma_start(out=cst_s[:, W1H:W1E], in_=cst[:, W1H:W1E])
            ksdma(nc.gpsimd, 5)
            ksdma(nc.scalar, 3)
            ksdma(nc.sync, 4)
            # pair-stage constants: selectors on pool, w2/w3/w4+b on scalar
            nc.gpsimd.dma_start(out=cst_s[:, CR:CR1], in_=cst[:, CR:CR1])
            nc.scalar.dma_start(out=cst_s[:, CR1:], in_=cst[:, CR1:])

            ORDER = [0, 1, 2, 5, 3, 4]   # expected arrival order
            first_k = True
            for oi, ci in enumerate(ORDER):
                CL = CHUNKS[ci] * NC
                kc = kcs[ci]
                sq = sqp.tile([P, 2, CL], bf16, tag=f"sq{ci}")
                nc.vector.tensor_mul(sq, kc, kc)
                m2 = m2p.tile([P, CL], bf16, tag=f"m2{ci}")
                nc.vector.tensor_add(m2, sq[:, 0, :], sq[:, 1, :])
                fc = f_s[:, koff[ci] * NC:koff[ci + 1] * NC]
                nc.scalar.sqrt(fc, m2)
                for k in range(koff[ci], koff[ci + 1]):
                    nc.tensor.matmul(
                        out=psum1,
                        lhsT=cst_s[:, 50 * k:50 * (k + 1)],
                        rhs=f_s[:, k * NC:(k + 1) * NC],
                        start=first_k,
                        stop=(oi == NCHUNK - 1 and k == koff[ci + 1] - 1),
                    )
                    first_k = False
            LASTC = ORDER[-1]
            # PE heater: tiny junk matmuls anchored on the last chunk's
            # sqrt output keep the HAM activity window hot across the
            # stream->tail gap so the pair-stage matmuls run at full clock
            junk = psp.tile([1, 4], f32, tag="jnk")
            fanchor = f_s[0:1, koff[LASTC + 1] * NC - 4:koff[LASTC + 1] * NC]
            for _ in range(24):
                nc.tensor.matmul(out=junk,
                                 lhsT=cst_s[0:1, o["one"]:o["one"] + 1],
                                 rhs=fanchor, start=True, stop=True)
            # swap ACT to the sigmoid table set; anchor the dummy on the
            # last sqrt output so the scheduler cannot hoist it earlier.
            # relu/identity live in the sigmoid set too, so the whole tail
            # runs without another table load.
            nc.scalar.activation(
                out=prim[:, 1:2],
                in_=f_s[0:1, koff[LASTC + 1] * NC - 1:koff[LASTC + 1] * NC],
                func=AF.Sigmoid)

            # ---- finalize column projections ----
            # one cast of the whole psum1 block to bf16
            paq = mlp.tile([50, NC], bf16, tag="paq")
            nc.vector.tensor_copy(paq, psum1)

            # pa4[128, NWC] = quadrant-replicated Pa, then + b1 via a
            # rank-1 matmul (b1 pattern row x ones row)
            pa4 = psp.tile([P, NWC], f32, tag="pa4")
            nc.tensor.matmul(out=pa4,
                             lhsT=cst_s[0:CHANS, o["sel"]:o["sel"] + P],
                             rhs=paq[0:CHANS, NS:NC], start=True, stop=False)
            nc.tensor.matmul(out=pa4,
                             lhsT=cst_s[0:1, o["b1p"]:o["b1p"] + P],
                             rhs=cst_s[0:1, o["one"]:o["one"] + NWC],
                             start=False, stop=True)
            # pq4[128, NL]: quadrant j rows 32j..32j+17 get Pq[:, j*NL+l].
            # Full-width sel32 writes quadrant 3 (its values land in every
            # quadrant but 0..2 are overwritten right after by 32-col
            # slices of the same selector at legal output bases).
            pq4p = psp.tile([P, NL], f32, tag="pq4p")
            nc.tensor.matmul(
                out=pq4p, lhsT=cst_s[32:32 + CHANS, o["sel32"]:o["sel32"] + P],
                rhs=paq[32:32 + CHANS, 3 * NL:4 * NL], start=True, stop=True)
            for j in range(3):
                nc.tensor.matmul(
                    out=pq4p[32 * j:32 * (j + 1), :],
                    lhsT=cst_s[32:32 + CHANS,
                               o["sel32"] + 32 * j:o["sel32"] + 32 * (j + 1)],
                    rhs=paq[32:32 + CHANS, j * NL:(j + 1) * NL],
                    start=True, stop=True)
            bf32 = mlp.tile([P, 2], f32, tag="bf32")
            nc.vector.tensor_copy(bf32, cst_s[:, o["b2"]:o["b2"] + 2])

            # ---- pair MLP: h1 split across DVE and ACT, then the
            # W2..sigmoid chain pipelined in two wi-slot halves ----
            pq4b = mlp.tile([P, NL], bf16, tag="pq4b")
            nc.vector.tensor_copy(pq4b, pq4p)
            h1p = mlp.tile([P, NF], bf16, tag="h1p")
            scr = mlp.tile([4, NF], bf16, tag="scr")
            psum2f = psp.tile([P, NF], f32, tag="ps2")
            psum3f = psp.tile([P, NF], f32, tag="ps3")
            psum4f = psp.tile([4, NF], f32, tag="ps4")
            NLH = max(NL // 2, 1)
            halves = [(0, NLH), (NLH, NL)] if NL > 1 else [(0, 1)]
            for hi, (l0, l1) in enumerate(halves):
                for lw in range(l0, l1):
                    if hi == 0:
                        nc.vector.tensor_scalar(
                            out=h1p[:, lw * NWC:(lw + 1) * NWC], in0=pa4,
                            scalar1=pq4p[:, lw:lw + 1], scalar2=0.0,
                            op0=ALU.add, op1=ALU.max)
                    else:
                        nc.scalar.activation(
                            out=h1p[:, lw * NWC:(lw + 1) * NWC], in_=pa4,
                            func=AF.Relu, bias=pq4b[:, lw:lw + 1], scale=1.0)
            for _ in range(10):
                nc.tensor.matmul(out=junk,
                                 lhsT=cst_s[0:1, o["one"]:o["one"] + 1],
                                 rhs=h1p[0:1, 0:4], start=True, stop=True)
            for hi, (l0, l1) in enumerate(halves):
                HF = (l1 - l0) * NWC
                off = l0 * NWC
                psum2 = psum2f[:, off:off + HF]
                nc.tensor.matmul(out=psum2, lhsT=cst_s[:, o["w2"]:o["w2"] + P],
                                 rhs=h1p[:, off:off + HF], start=True, stop=True)
                h2p = mlp.tile([P, HF], bf16, tag=f"h2p{hi}")
                if hi == 0:
                    nc.vector.tensor_scalar(out=h2p, in0=psum2,
                                            scalar1=bf32[:, 0:1], scalar2=0.0,
                                            op0=ALU.add, op1=ALU.max)
                else:
                    nc.scalar.activation(out=h2p, in_=psum2, func=AF.Relu,
                                         bias=b2_s, scale=1.0)
                psum3 = psum3f[:, off:off + HF]
                nc.tensor.matmul(out=psum3, lhsT=cst_s[:, o["w3"]:o["w3"] + P],
                                 rhs=h2p, start=True, stop=True)
                h3p = mlp.tile([P, HF], bf16, tag=f"h3p{hi}")
                if hi == 0:
                    nc.vector.tensor_scalar(out=h3p, in0=psum3,
                                            scalar1=bf32[:, 1:2], scalar2=0.0,
                                            op0=ALU.add, op1=ALU.max)
                else:
                    nc.scalar.activation(out=h3p, in_=psum3, func=AF.Relu,
                                         bias=b3_s, scale=1.0)
                psum4 = psum4f[:, off:off + HF]
                nc.tensor.matmul(out=psum4, lhsT=cst_s[:, o["w4"]:o["w4"] + 4],
                                 rhs=h3p, start=True, stop=True)
                nc.scalar.activation(out=scr[:, off:off + HF], in_=psum4,
                                     func=AF.Sigmoid, bias=b4_s, scale=1.0)
            nc.sync.dma_start(out=hp[:], in_=scr)

    nc.finalize()
    return nc


def _run_sim(nc, in_maps):
    """CoreSim (CPU instruction simulator) path for local dev testing."""
    from concourse.bass_interp import MultiCoreSim
    from concourse.bass_utils import BassKernelResults

    sim = MultiCoreSim(nc, num_cores=len(in_maps))
    for core_id, core in sim.cores.items():
        for name, arr in in_maps[core_id].items():
            core.tensor(name)[:] = arr
    sim.simulate()
    results = [
        {"hp": np.array(sim.cores[i].tensor("hp"))} for i in range(len(in_maps))
    ]
    return BassKernelResults(results=results, instructions_and_trace=None,
                             profile_json=None, exec_time_ns=None)


def _mask_geometry(acquired_mask, acquiring_mask):
    """Replicates the reference's left/right/cmask/denom logic exactly."""
    am = np.asarray(acquired_mask, np.float32)
    qm = np.asarray(acquiring_mask, np.float32)
    mid = W // 2
    right = mid + np.argmax(am[:, mid:] < 1.0, axis=1)
    left = np.argmax(am[:, :mid][:, ::-1] < 1.0, axis=1) + 1
    cols = np.arange(W)
    cmask = (cols[None, :] >= left[:, None]) & (cols[None, :] < right[:, None])
    denom = (right - left).astype(np.float32)
    active = [np.nonzero(qm[b] > 0)[0] for b in range(B)]
    return left.astype(int), right.astype(int), cmask, denom, active


def kernel(acquired_kspace, acquiring_kspace, acquired_mask, acquiring_mask,
           W1, b1, W2, b2, W3, b3, W4, b4):
    global LAST_RESULTS
    from concourse.bass_utils import run_bass_kernel_spmd

    acquired_kspace = np.asarray(acquired_kspace, np.float32)
    acquiring_kspace = np.asarray(acquiring_kspace, np.float32)
    W1 = np.asarray(W1, np.float32)
    b1 = np.asarray(b1, np.float32)
    W2 = np.asarray(W2, np.float32)
    b2 = np.asarray(b2, np.float32)
    W3 = np.asarray(W3, np.float32)
    b3 = np.asarray(b3, np.float32)
    W4 = np.asarray(W4, np.float32)
    b4 = np.asarray(b4, np.float32)

    left, right, cmask, denom, active = _mask_geometry(acquired_mask, acquiring_mask)

    nmax = max(len(a) for a in active)
    out = np.zeros((B, H, W), np.float32)
    if nmax == 0:
        return out

    span = max(int((right - left).max()), 1)
    NL = max(1, math.ceil(nmax / 4))          # wi slots per quadrant
    NWC = max(1, math.ceil(span / 2))         # wc columns per core
    NS = 4 * NL
    NC = NS + NWC
    assert NL * NWC <= 512, (NL, NWC)
    o = _cst_layout(NWC, NS)
    CW = o["end"]

    # ---- shared constant block [128, CW] bf16 ----
    W1q, W1a = W1[:, :CH], W1[:, CH:]
    cstv = np.zeros((P, CW), np.float32)
    # w1cat: per k-tile 50 cols; W1a_k at 0:18, W1q_k at 32:50 so both
    # PSUM row groups are 32-aligned for engine reads
    w1q_t = W1q.T.reshape(KT, P, CHANS)   # [k, p, i]
    w1a_t = W1a.T.reshape(KT, P, CHANS)
    w1cat = np.zeros((KT, P, 50), np.float32)
    w1cat[:, :, 0:CHANS] = w1a_t
    w1cat[:, :, 32:32 + CHANS] = w1q_t
    cstv[:, :KT * 50] = w1cat.transpose(1, 0, 2).reshape(P, -1)
    eye = np.eye(CHANS, dtype=np.float32)
    for j in range(4):
        sl = slice(32 * j, 32 * j + CHANS)
        cstv[:CHANS, o["sel"] + 32 * j:o["sel"] + 32 * j + CHANS] = eye
        cstv[32:32 + CHANS, o["sel32"] + 32 * j:
             o["sel32"] + 32 * j + CHANS] = eye
        cstv[sl, o["w2"] + 32 * j:o["w2"] + 32 * j + CHANS] = W2.T
        cstv[sl, o["w3"] + 32 * j:o["w3"] + 32 * j + CHANS] = W3.T
        cstv[sl, o["w4"] + j] = W4[0]
        cstv[sl, o["b2"]] = b2
        cstv[sl, o["b3"]] = b3
    for j in range(4):
        cstv[0, o["b1p"] + 32 * j:o["b1p"] + 32 * j + CHANS] = b1
    cstv[0, o["one"]:o["b2"]] = 1.0
    cstv[:4, o["b4"]] = float(b4[0])
    cstv = cstv.astype(BF16)

    # ---- per-core packed k-space [128, 2, KT*NC] bf16 ----
    # |z| features; ch = k*128 + p; combined cols = [q slots | a cols]
    in_maps = []
    meta = []
    for b in range(B):
        aw = active[b]
        awp = np.zeros(NS, np.int64)
        if len(aw):
            awp[:len(aw)] = aw
            awp[len(aw):] = aw[0]
        # [CH, NS, 2] -> re/im planes [2, KT, 128, NS]
        qcols = acquiring_kspace[b].reshape(CH, W, 2)[:, awp, :]
        for s in range(2):
            w0 = int(left[b]) + s * NWC
            w1e = max(min(w0 + NWC, W), w0)
            nreal = w1e - w0
            acols = np.zeros((CH, NWC, 2), np.float32)
            if nreal > 0:
                acols[:, :nreal, :] = (
                    acquired_kspace[b].reshape(CH, W, 2)[:, w0:w1e, :])
            comb = np.concatenate([qcols, acols], axis=1)   # [CH, NC, 2]
            # -> [p, r, k, c]
            ksv = comb.reshape(KT, P, NC, 2).transpose(1, 3, 0, 2)
            in_maps.append(dict(
                ks=np.ascontiguousarray(ksv.reshape(P, 2, KT * NC)).astype(BF16),
                cst=cstv))
            meta.append((b, s, NWC - nreal))
    key = (NWC, NL)
    if key not in _prog_cache:
        _prog_cache[key] = _build_program(NWC, NL)
    nc = _prog_cache[key]

    trace = bool(int(os.environ.get("CABSK_TRACE", "0")))
    tmpdir = os.environ.get("CABSK_TMPDIR") or None
    if tmpdir:
        import tempfile
        tmpdir = tempfile.mkdtemp(dir=tmpdir)
    if os.environ.get("CABSK_SIM", "0") == "1":
        res = _run_sim(nc, in_maps)
    else:
        res = run_bass_kernel_spmd(nc, in_maps, core_ids=list(range(NCORES)),
                                   trace=trace, tmpdir=tmpdir)
    LAST_RESULTS = res

    # ---- host epilogue: per-column sums, skipping padding columns ----
    heat = np.zeros((B, W), np.float32)
    for ci, (b, s, npad) in enumerate(meta):
        hpv = np.asarray(res.results[ci]["hp"], np.float32)   # [4, NL*NWC]
        hpv = hpv.reshape(4, NL, NWC)[:, :, :NWC - npad].sum(axis=2)
        aw = active[b]
        d = denom[b] if denom[b] != 0 else 1.0
        for t in range(len(aw)):
            heat[b, aw[t]] += hpv[t // NL, t % NL] / d
    out[:] = heat[:, None, :]
    return out
